# revision 51
# baseline (speedup 1.0000x reference)
"""MultiHeadGAT layer on 8 trn2 NeuronCores, data-parallel over batch.

Rank-1 softmax factorization removes per-element exp entirely:
  exp(leaky(s_ij)) = max(exp(s), exp(0.2 s)),   s = es_i + ed_j
Dividing by exp(0.2*es_i) (cancels between numerator and denominator) and
normalizing by e^{-M_h} (M_h = 0.8*max_i es, also cancels):
  P''[j,i] = max( r_i * v_j , q_j )
    r = exp(0.8*es_i)       broadcast over partitions (per head, via PE
                            one-hot selector matmul)
    v = exp(ed_j - M_h)     per-partition scalar
    q = exp(0.2*ed_j - M_h) per-partition scalar
  x = P'' * adjT;  AV matmul with a ones-column (aug) gives num rows 0..63
  and the softmax denominator in row 64.  num/den division happens on host
  (any per-i factor cancels there too).

E (the [16, n] src/dst projections h @ W a) is precomputed on host - it is
0.03% of the FLOPs but gates the entire startup dependence chain.

Hot loop runs QUAD-grouped: 4 heads iterate jb together so (a) each adjT
tile is consumed 4x slower than DMA delivers it, (b) ONE fused tensor_mul
[128, 4x1024] applies the mask for all 4 heads (adjT free-dim-broadcast
AP), (c) 4 heads' accumulators live in PSUM at once (8 banks, enabled by
closing the setup PSUM pool scope before the hot loop opens its own).
Per (head, jb) segment: DVE tensor_scalar (mult,max) or - on ~20 A2 tiles
to balance engines - ACT relu(rv-q) then relu(+q).  fp16 throughout the
hot path (beats bf16 ~20% on DVE/ACT here); fp32 accumulate.
SP dma_start dispatch costs ~600ns serially, so loads are issued in
need-by order and split so per-partition descriptors spread across queues.
"""
import sys

sys.path.insert(0, "/opt/trn_rl_repo")

import numpy as np

import concourse.bass as bass
import concourse.mybir as mybir
import concourse.tile as tile
from concourse.bass_utils import run_bass_kernel_spmd
from concourse.masks import make_identity

F32 = mybir.dt.float32
FP16 = mybir.dt.float16
AF = mybir.ActivationFunctionType
ALU = mybir.AluOpType

N_CORES = 8
N = 1024
NB = 8          # row blocks of 128
FIN = 256
KT = 2          # FIN / 128
FO = 512        # heads * fo
H = 8
FOH = 64
ALPHA = 0.2

# A2 tiles: (quad-group, jb) -> heads whose max(rv,q) runs on ACT.
# <=2 per quad-tile (ACT must keep pace with the fused tt), none during
# the first tiles of qg0 while ACT still stages rbrd/aug.
A2Q = {
    (0, 5): (1,), (0, 6): (2,), (0, 7): (3,),
    (1, 0): (4,), (1, 1): (5,), (1, 2): (6,), (1, 3): (7,),
    (1, 4): (4, 5), (1, 5): (6, 7),
}

_MAX_SYNC_WAITS = 1


def _split_sync_waits(nc, max_waits=_MAX_SYNC_WAITS):
    """This walrus build rejects instructions carrying more than one sync
    wait; hoist extras onto NOPs inserted just before, on the same engine."""
    uid = 0
    for f in nc.m.functions:
        for bb in f.blocks:
            out = []
            for inst in bb.instructions:
                si = getattr(inst, "sync_info", None)
                if si is not None and si.on_wait and len(si.on_wait) > max_waits:
                    waits = list(si.on_wait)
                    keep = waits[-max_waits:]
                    extra = waits[:-max_waits]
                    si.on_wait.clear()
                    si.on_wait.extend(keep)
                    while extra:
                        chunk, extra = extra[:max_waits], extra[max_waits:]
                        nop = mybir.InstNoOp(
                            name=f"waitsplit-{uid}",
                            engine=inst.engine,
                            sync_info=mybir.SyncInfo(
                                on_wait=list(chunk), on_update=[]
                            ),
                            bass_nofuse=True,
                        )
                        uid += 1
                        out.append(nop)
                out.append(inst)
            bb.instructions[:] = out


def _dma_split(nc, dst, src, parts):
    """Issue a tile load/store as `parts` dma_starts so the per-partition
    descriptors spread across DMA queues instead of serializing on one."""
    p = dst.shape[0]
    step = (p + parts - 1) // parts
    for i in range(0, p, step):
        j = min(i + step, p)
        nc.sync.dma_start(dst[i:j], src[i:j])


def build_nc(split=True):
    nc = bass.Bass()
    ht_d = nc.declare_dram_parameter("hT", [FIN, N], FP16, isOutput=False)
    adjt_d = nc.declare_dram_parameter("adjT", [N, N], FP16, isOutput=False)
    w_d = nc.declare_dram_parameter("Wp", [128, 2 * FO], FP16, isOutput=False)
    e_d = nc.declare_dram_parameter("R", [16, N], FP16, isOutput=False)
    vq_d = nc.declare_dram_parameter("VQ", [32, N], F32, isOutput=False)
    out_d = nc.declare_dram_parameter("out", [H * 65, N], F32, isOutput=True)

    with tile.TileContext(nc) as tc:
        with (
            tc.tile_pool(name="const", bufs=1) as const,
            tc.tile_pool(name="persist", bufs=1) as persist,
            tc.tile_pool(name="x1p", bufs=4) as x1p,
            tc.tile_pool(name="epi", bufs=4) as epi,
        ):
            # ---- input loads in need-by order.  r/v/q are the host-side
            # exps of the [16, n] projections (removes the ACT exp-table
            # load + exp chain from the critical path). ----
            adjT = [persist.tile([128, N], FP16, tag=f"adjT{j}", name=f"adjT{j}")
                    for j in range(NB)]
            _dma_split(nc, adjT[0][:], adjt_d[0:128, :], 2)
            r_t = const.tile([16, N], FP16, tag="rT")
            nc.sync.dma_start(r_t[:], e_d[0:16, :])
            v_t = const.tile([16, N], F32, tag="vT")
            nc.sync.dma_start(v_t[:], vq_d[0:16, :])
            q_t = const.tile([16, N], F32, tag="qT")
            nc.sync.dma_start(q_t[:], vq_d[16:32, :])
            _dma_split(nc, adjT[1][:], adjt_d[128:256, :], 2)
            hT = []
            for k in range(KT):
                t = const.tile([128, N], FP16, tag=f"hT{k}", name=f"hT{k}")
                _dma_split(nc, t[:], ht_d[k * 128:(k + 1) * 128, :], 2)
                hT.append(t)
            wp = const.tile([128, 2 * FO], FP16, tag="Wp")
            _dma_split(nc, wp[:], w_d[:, :], 2)
            wk = [wp[:, k * FO:(k + 1) * FO] for k in range(KT)]
            for jb in range(2, NB):
                _dma_split(nc, adjT[jb][:], adjt_d[jb * 128:(jb + 1) * 128, :], 2)

            ident = const.tile([128, 128], F32, tag="ident")
            make_identity(nc, ident[:])

            # one-hot selector rows for the r broadcast: sel[hh][k, m]=d(k,hh)
            sel = []
            for hh in range(H):
                t = const.tile([16, 128], FP16, tag=f"sel{hh}", name=f"sel{hh}")
                nc.gpsimd.memset(t[:], 0.0)
                nc.gpsimd.affine_select(
                    out=t[:], in_=t[:], pattern=[[0, 128]],
                    compare_op=mybir.AluOpType.not_equal, fill=1.0,
                    base=-hh, channel_multiplier=1,
                )
                sel.append(t)

            vq_sb = [persist.tile([128, 32], F32, tag=f"vq{j}", name=f"vq{j}")
                     for j in range(NB)]
            nq_sb = [persist.tile([128, 8], F32, tag=f"nq{j}", name=f"nq{j}")
                     for j in range(NB)]
            rbrd = [persist.tile([128, N], FP16, tag=f"rb{hh}", name=f"rb{hh}")
                    for hh in range(H)]
            wh_aug = [persist.tile([128, H * 65], FP16, tag=f"wha{j}", name=f"wha{j}")
                      for j in range(NB)]

            # ---- setup PSUM scope: closed before the hot loop so psAcc
            # can take all 8 banks ----
            with tc.tile_pool(name="psS", bufs=3, space="PSUM") as psS:
                def rbrd_build(hh):
                    for c in range(2):
                        ps = psS.tile([128, 512], F32, tag="ps")
                        nc.tensor.matmul(
                            ps[:], sel[hh][:], r_t[:, c * 512:(c + 1) * 512],
                            start=True, stop=True,
                        )
                        nc.scalar.copy(
                            rbrd[hh][:, c * 512:(c + 1) * 512], ps[:]
                        )

                # vq first: PSUM tiles drain fast via idle DVE
                for jb in range(NB):
                    ps = psS.tile([128, 512], F32, tag="ps")
                    nc.tensor.transpose(
                        ps[:, 0:16], v_t[:, jb * 128:(jb + 1) * 128],
                        ident[0:16, 0:16],
                    )
                    nc.tensor.transpose(
                        ps[:, 16:32], q_t[:, jb * 128:(jb + 1) * 128],
                        ident[0:16, 0:16],
                    )
                    nc.vector.tensor_copy(vq_sb[jb][:], ps[:, 0:32])
                    nc.vector.tensor_scalar_mul(
                        nq_sb[jb][:], vq_sb[jb][:, 24:32], -1.0
                    )

                for hh in range(4):
                    rbrd_build(hh)
                for jb in range(NB):
                    ps = psS.tile([128, 512], F32, tag="ps")
                    for k in range(KT):
                        nc.tensor.matmul(
                            ps[:], hT[k][:, jb * 128:(jb + 1) * 128], wk[k],
                            start=(k == 0), stop=(k == KT - 1),
                        )
                    aug3 = wh_aug[jb][:].rearrange("p (h f) -> p h f", h=H)
                    ps3 = ps[:].rearrange("p (h f) -> p h f", f=FOH)
                    nc.gpsimd.memset(aug3[:, :, FOH:FOH + 1], 1.0)
                    nc.scalar.activation(aug3[:, :, 0:FOH], ps3, AF.Copy)
                for hh in range(4, H):
                    rbrd_build(hh)

            # ---- hot loop: two quad-groups of 4 heads.  The qg0
            # epilogues are emitted INSIDE qg1's stream so ACT's serial
            # queue never stalls the A2/mask dependency chain. ----
            with tc.tile_pool(name="psA", bufs=1, space="PSUM") as psA:
                def epi_emit(hh, acc_t, dve=False, dma_eng=None):
                    eng = dma_eng if dma_eng is not None else nc.sync
                    acc_sb = epi.tile([65, N], F32, tag="accsb")
                    for c in range(2):
                        if dve:
                            nc.vector.tensor_copy(
                                acc_sb[:, c * 512:(c + 1) * 512],
                                acc_t[:, c * 512:(c + 1) * 512],
                            )
                        else:
                            nc.scalar.copy(
                                acc_sb[:, c * 512:(c + 1) * 512],
                                acc_t[:, c * 512:(c + 1) * 512],
                            )
                        dst = out_d[hh * 65:(hh + 1) * 65,
                                    c * 512:(c + 1) * 512]
                        src = acc_sb[:, c * 512:(c + 1) * 512]
                        eng.dma_start(dst[0:33], src[0:33])
                        eng.dma_start(dst[33:65], src[33:65])

                prev_acc = {}
                for qg in range(2):
                    hs = [4 * qg + i for i in range(4)]
                    acc = [psA.tile([65, N], F32, tag=f"acc{i}", name=f"acc{i}")
                           for i in range(4)]
                    for jb in range(NB):
                        a2 = A2Q.get((qg, jb), ())
                        v_col = vq_sb[jb]
                        z4 = x1p.tile([128, 4 * N], FP16, tag="z")
                        for i, hh in enumerate(hs):
                            seg = z4[:, i * N:(i + 1) * N]
                            v_ap = v_col[:, 8 + hh:9 + hh]
                            q_ap = v_col[:, 24 + hh:25 + hh]
                            if hh in a2:
                                nq_ap = nq_sb[jb][:, hh:hh + 1]
                                nc.scalar.activation(
                                    seg, rbrd[hh][:], AF.Relu,
                                    bias=nq_ap, scale=v_ap,
                                )
                                nc.scalar.activation(
                                    seg, seg, AF.Relu, bias=q_ap
                                )
                            else:
                                nc.vector.tensor_scalar(
                                    seg, rbrd[hh][:], v_ap, q_ap,
                                    ALU.mult, ALU.max,
                                )
                        x4 = x1p.tile([128, 4 * N], FP16, tag="x")
                        adj4 = (adjT[jb][:]
                                .rearrange("p (o n) -> p o n", o=1)
                                .broadcast_to([128, 4, N]))
                        nc.vector.tensor_mul(
                            x4[:].rearrange("p (o n) -> p o n", o=4),
                            z4[:].rearrange("p (o n) -> p o n", o=4),
                            adj4,
                        )
                        for i, hh in enumerate(hs):
                            for c in range(2):
                                nc.tensor.matmul(
                                    acc[i][:, c * 512:(c + 1) * 512],
                                    wh_aug[jb][:, hh * 65:(hh + 1) * 65],
                                    x4[:, i * N + c * 512:i * N + (c + 1) * 512],
                                    start=(jb == 0), stop=(jb == NB - 1),
                                )
                        if qg == 1 and jb < 4:
                            h_prev, acc_prev = prev_acc.pop(jb)
                            epi_emit(h_prev, acc_prev)
                    if qg == 0:
                        prev_acc = {i: (hs[i], acc[i]) for i in range(4)}
                for i, hh in enumerate(hs):
                    epi_emit(hh, acc[i], dve=(i % 2 == 1),
                             dma_eng=nc.scalar if i % 2 == 0 else nc.sync)

    if split:
        _split_sync_waits(nc)
    return nc


_NC_CACHE = None


def _get_nc():
    global _NC_CACHE
    if _NC_CACHE is None:
        _NC_CACHE = build_nc()
    return _NC_CACHE


_NPDT = np.dtype(mybir.dt.np(FP16))


def _prep_in_maps(h, adj, W, a):
    h = np.asarray(h, dtype=np.float32)
    adj = np.asarray(adj)
    W = np.asarray(W, dtype=np.float32)
    a = np.asarray(a, dtype=np.float32)
    amat = np.zeros((FO, 2 * H), dtype=np.float32)
    for hh in range(H):
        amat[hh * FOH:(hh + 1) * FOH, hh] = a[hh, :FOH]
        amat[hh * FOH:(hh + 1) * FOH, H + hh] = a[hh, FOH:]
    wamat = W @ amat                       # [FIN, 16] fp32
    wp = np.ascontiguousarray(
        np.concatenate([W[0:128, :], W[128:256, :]], axis=1), dtype=_NPDT
    )                                      # [128, 1024] packed (2KB rows)
    in_maps = []
    for c in range(N_CORES):
        ee = (h[c] @ wamat).T              # [16, N] fp32: rows 0..7 es, 8..15 ed
        es, ed = ee[0:8], ee[8:16]
        mh = 0.8 * es.max(axis=1, keepdims=True)     # [8, 1]
        rr = np.zeros((16, N), dtype=_NPDT)
        rr[0:8] = np.exp(0.8 * es)
        vq = np.zeros((32, N), dtype=np.float32)
        vq[8:16] = np.exp(ed - mh)
        vq[24:32] = np.exp(ALPHA * ed - mh)
        in_maps.append({
            "hT": np.ascontiguousarray(h[c].T, dtype=_NPDT),
            "adjT": np.ascontiguousarray(adj[c].T, dtype=_NPDT),
            "Wp": wp,
            "R": rr,
            "VQ": vq,
        })
    return in_maps


def run(h, adj, W, a, trace=False, **kw):
    nc = _get_nc()
    in_maps = _prep_in_maps(h, adj, W, a)
    res = run_bass_kernel_spmd(nc, in_maps, list(range(N_CORES)), trace=trace, **kw)
    out = np.empty((N_CORES, N, FO), dtype=np.float32)
    for c in range(N_CORES):
        arr = res.results[c]["out"].reshape(H, 65, N)
        num = arr[:, :FOH, :]              # [H, 64, N]
        den = arr[:, FOH, :]               # [H, N]
        out[c] = (num / den[:, None, :]).transpose(2, 0, 1).reshape(N, FO)
    return out, res


def kernel(h, adj, W, a):
    out, _ = run(h, adj, W, a)
    return out


# revision 52
# speedup vs baseline: 1.0240x; 1.0240x over previous
"""MultiHeadGAT layer on 8 trn2 NeuronCores, data-parallel over batch.

Rank-1 softmax factorization removes per-element exp entirely:
  exp(leaky(s_ij)) = max(exp(s), exp(0.2 s)),   s = es_i + ed_j
Dividing by exp(0.2*es_i) (cancels between numerator and denominator) and
normalizing by e^{-M_h} (M_h = 0.8*max_i es, also cancels):
  P''[j,i] = max( r_i * v_j , q_j )
    r = exp(0.8*es_i)       broadcast over partitions (per head, via PE
                            one-hot selector matmul)
    v = exp(ed_j - M_h)     per-partition scalar
    q = exp(0.2*ed_j - M_h) per-partition scalar
  x = P'' * adjT;  AV matmul with a ones-column (aug) gives num rows 0..63
  and the softmax denominator in row 64.  num/den division happens on host
  (any per-i factor cancels there too).

E (the [16, n] src/dst projections h @ W a) is precomputed on host - it is
0.03% of the FLOPs but gates the entire startup dependence chain.

Hot loop runs QUAD-grouped: 4 heads iterate jb together so (a) each adjT
tile is consumed 4x slower than DMA delivers it, (b) ONE fused tensor_mul
[128, 4x1024] applies the mask for all 4 heads (adjT free-dim-broadcast
AP), (c) 4 heads' accumulators live in PSUM at once (8 banks, enabled by
closing the setup PSUM pool scope before the hot loop opens its own).
Per (head, jb) segment: DVE tensor_scalar (mult,max) or - on ~20 A2 tiles
to balance engines - ACT relu(rv-q) then relu(+q).  fp16 throughout the
hot path (beats bf16 ~20% on DVE/ACT here); fp32 accumulate.
SP dma_start dispatch costs ~600ns serially, so loads are issued in
need-by order and split so per-partition descriptors spread across queues.
"""
import sys

sys.path.insert(0, "/opt/trn_rl_repo")

import numpy as np

import concourse.bass as bass
import concourse.mybir as mybir
import concourse.tile as tile
from concourse.bass_utils import run_bass_kernel_spmd
from concourse.masks import make_identity

F32 = mybir.dt.float32
FP16 = mybir.dt.float16
AF = mybir.ActivationFunctionType
ALU = mybir.AluOpType

N_CORES = 8
N = 1024
NB = 8          # row blocks of 128
FIN = 256
KT = 2          # FIN / 128
FO = 512        # heads * fo
H = 8
FOH = 64
ALPHA = 0.2

# A2 tiles: (quad-group, jb) -> heads whose max(rv,q) runs on ACT.
# <=2 per quad-tile (ACT must keep pace with the fused tt), none during
# the first tiles of qg0 while ACT still stages rbrd/aug.
A2Q = {
    (0, 5): (1,), (0, 6): (2,), (0, 7): (3,),
    (1, 2): (4,), (1, 3): (5, 6), (1, 4): (7,),
    (1, 5): (4,), (1, 6): (5, 6), (1, 7): (7,),
}

_MAX_SYNC_WAITS = 1


def _split_sync_waits(nc, max_waits=_MAX_SYNC_WAITS):
    """This walrus build rejects instructions carrying more than one sync
    wait; hoist extras onto NOPs inserted just before, on the same engine."""
    uid = 0
    for f in nc.m.functions:
        for bb in f.blocks:
            out = []
            for inst in bb.instructions:
                si = getattr(inst, "sync_info", None)
                if si is not None and si.on_wait and len(si.on_wait) > max_waits:
                    waits = list(si.on_wait)
                    keep = waits[-max_waits:]
                    extra = waits[:-max_waits]
                    si.on_wait.clear()
                    si.on_wait.extend(keep)
                    while extra:
                        chunk, extra = extra[:max_waits], extra[max_waits:]
                        nop = mybir.InstNoOp(
                            name=f"waitsplit-{uid}",
                            engine=inst.engine,
                            sync_info=mybir.SyncInfo(
                                on_wait=list(chunk), on_update=[]
                            ),
                            bass_nofuse=True,
                        )
                        uid += 1
                        out.append(nop)
                out.append(inst)
            bb.instructions[:] = out


def _dma_split(nc, dst, src, parts):
    """Issue a tile load/store as `parts` dma_starts so the per-partition
    descriptors spread across DMA queues instead of serializing on one."""
    p = dst.shape[0]
    step = (p + parts - 1) // parts
    for i in range(0, p, step):
        j = min(i + step, p)
        nc.sync.dma_start(dst[i:j], src[i:j])


def build_nc(split=True):
    nc = bass.Bass()
    ht_d = nc.declare_dram_parameter("hT", [FIN, N], FP16, isOutput=False)
    adjt_d = nc.declare_dram_parameter("adjT", [N, N], FP16, isOutput=False)
    w_d = nc.declare_dram_parameter("Wp", [128, 2 * FO], FP16, isOutput=False)
    e_d = nc.declare_dram_parameter("R", [16, N], FP16, isOutput=False)
    vq_d = nc.declare_dram_parameter("VQ", [32, N], F32, isOutput=False)
    out_d = nc.declare_dram_parameter("out", [H * 65, N], F32, isOutput=True)

    with tile.TileContext(nc) as tc:
        with (
            tc.tile_pool(name="const", bufs=1) as const,
            tc.tile_pool(name="persist", bufs=1) as persist,
            tc.tile_pool(name="x1p", bufs=4) as x1p,
            tc.tile_pool(name="epi", bufs=4) as epi,
        ):
            # ---- input loads in need-by order.  r/v/q are the host-side
            # exps of the [16, n] projections (removes the ACT exp-table
            # load + exp chain from the critical path). ----
            adjT = [persist.tile([128, N], FP16, tag=f"adjT{j}", name=f"adjT{j}")
                    for j in range(NB)]
            _dma_split(nc, adjT[0][:], adjt_d[0:128, :], 2)
            r_t = const.tile([16, N], FP16, tag="rT")
            nc.sync.dma_start(r_t[:], e_d[0:16, :])
            v_t = const.tile([16, N], F32, tag="vT")
            nc.sync.dma_start(v_t[:], vq_d[0:16, :])
            q_t = const.tile([16, N], F32, tag="qT")
            nc.sync.dma_start(q_t[:], vq_d[16:32, :])
            _dma_split(nc, adjT[1][:], adjt_d[128:256, :], 2)
            hT = []
            for k in range(KT):
                t = const.tile([128, N], FP16, tag=f"hT{k}", name=f"hT{k}")
                _dma_split(nc, t[:], ht_d[k * 128:(k + 1) * 128, :], 2)
                hT.append(t)
            wp = const.tile([128, 2 * FO], FP16, tag="Wp")
            _dma_split(nc, wp[:], w_d[:, :], 2)
            wk = [wp[:, k * FO:(k + 1) * FO] for k in range(KT)]
            for jb in range(2, NB):
                _dma_split(nc, adjT[jb][:], adjt_d[jb * 128:(jb + 1) * 128, :], 2)

            ident = const.tile([128, 128], F32, tag="ident")
            make_identity(nc, ident[:])

            # one-hot selector rows for the r broadcast: sel[hh][k, m]=d(k,hh)
            sel = []
            for hh in range(H):
                t = const.tile([16, 128], FP16, tag=f"sel{hh}", name=f"sel{hh}")
                nc.gpsimd.memset(t[:], 0.0)
                nc.gpsimd.affine_select(
                    out=t[:], in_=t[:], pattern=[[0, 128]],
                    compare_op=mybir.AluOpType.not_equal, fill=1.0,
                    base=-hh, channel_multiplier=1,
                )
                sel.append(t)

            vq_sb = [persist.tile([128, 32], F32, tag=f"vq{j}", name=f"vq{j}")
                     for j in range(NB)]
            nq_sb = [persist.tile([128, 8], F32, tag=f"nq{j}", name=f"nq{j}")
                     for j in range(NB)]
            rbrd = [persist.tile([128, N], FP16, tag=f"rb{hh}", name=f"rb{hh}")
                    for hh in range(H)]
            wh_aug = [persist.tile([128, H * 65], FP16, tag=f"wha{j}", name=f"wha{j}")
                      for j in range(NB)]

            # ---- setup PSUM scope: closed before the hot loop so psAcc
            # can take all 8 banks ----
            with tc.tile_pool(name="psS", bufs=3, space="PSUM") as psS:
                def rbrd_build(hh):
                    for c in range(2):
                        ps = psS.tile([128, 512], F32, tag="ps")
                        nc.tensor.matmul(
                            ps[:], sel[hh][:], r_t[:, c * 512:(c + 1) * 512],
                            start=True, stop=True,
                        )
                        nc.scalar.copy(
                            rbrd[hh][:, c * 512:(c + 1) * 512], ps[:]
                        )

                # vq first: PSUM tiles drain fast via idle DVE
                for jb in range(NB):
                    ps = psS.tile([128, 512], F32, tag="ps")
                    nc.tensor.transpose(
                        ps[:, 0:16], v_t[:, jb * 128:(jb + 1) * 128],
                        ident[0:16, 0:16],
                    )
                    nc.tensor.transpose(
                        ps[:, 16:32], q_t[:, jb * 128:(jb + 1) * 128],
                        ident[0:16, 0:16],
                    )
                    nc.vector.tensor_copy(vq_sb[jb][:], ps[:, 0:32])
                    nc.vector.tensor_scalar_mul(
                        nq_sb[jb][:], vq_sb[jb][:, 24:32], -1.0
                    )

                for hh in range(4):
                    rbrd_build(hh)
                for jb in range(NB):
                    ps = psS.tile([128, 512], F32, tag="ps")
                    for k in range(KT):
                        nc.tensor.matmul(
                            ps[:], hT[k][:, jb * 128:(jb + 1) * 128], wk[k],
                            start=(k == 0), stop=(k == KT - 1),
                        )
                    aug3 = wh_aug[jb][:].rearrange("p (h f) -> p h f", h=H)
                    ps3 = ps[:].rearrange("p (h f) -> p h f", f=FOH)
                    nc.gpsimd.memset(aug3[:, :, FOH:FOH + 1], 1.0)
                    nc.scalar.activation(aug3[:, :, 0:FOH], ps3, AF.Copy)
                for hh in range(4, H):
                    rbrd_build(hh)

            # ---- hot loop: two quad-groups of 4 heads.  The qg0
            # epilogues are emitted INSIDE qg1's stream so ACT's serial
            # queue never stalls the A2/mask dependency chain. ----
            with tc.tile_pool(name="psA", bufs=1, space="PSUM") as psA:
                def epi_emit(hh, acc_t, dve=False, dma_eng=None):
                    eng = dma_eng if dma_eng is not None else nc.sync
                    acc_sb = epi.tile([65, N], F32, tag="accsb")
                    for c in range(2):
                        if dve:
                            nc.vector.tensor_copy(
                                acc_sb[:, c * 512:(c + 1) * 512],
                                acc_t[:, c * 512:(c + 1) * 512],
                            )
                        else:
                            nc.scalar.copy(
                                acc_sb[:, c * 512:(c + 1) * 512],
                                acc_t[:, c * 512:(c + 1) * 512],
                            )
                        dst = out_d[hh * 65:(hh + 1) * 65,
                                    c * 512:(c + 1) * 512]
                        src = acc_sb[:, c * 512:(c + 1) * 512]
                        eng.dma_start(dst[0:33], src[0:33])
                        eng.dma_start(dst[33:65], src[33:65])

                prev_acc = {}
                for qg in range(2):
                    hs = [4 * qg + i for i in range(4)]
                    acc = [psA.tile([65, N], F32, tag=f"acc{i}", name=f"acc{i}")
                           for i in range(4)]
                    for jb in range(NB):
                        a2 = A2Q.get((qg, jb), ())
                        v_col = vq_sb[jb]
                        z4 = x1p.tile([128, 4 * N], FP16, tag="z")
                        for i, hh in enumerate(hs):
                            seg = z4[:, i * N:(i + 1) * N]
                            v_ap = v_col[:, 8 + hh:9 + hh]
                            q_ap = v_col[:, 24 + hh:25 + hh]
                            if hh in a2:
                                nq_ap = nq_sb[jb][:, hh:hh + 1]
                                nc.scalar.activation(
                                    seg, rbrd[hh][:], AF.Relu,
                                    bias=nq_ap, scale=v_ap,
                                )
                                nc.scalar.activation(
                                    seg, seg, AF.Relu, bias=q_ap
                                )
                            else:
                                nc.vector.tensor_scalar(
                                    seg, rbrd[hh][:], v_ap, q_ap,
                                    ALU.mult, ALU.max,
                                )
                        x4 = x1p.tile([128, 4 * N], FP16, tag="x")
                        adj4 = (adjT[jb][:]
                                .rearrange("p (o n) -> p o n", o=1)
                                .broadcast_to([128, 4, N]))
                        nc.vector.tensor_mul(
                            x4[:].rearrange("p (o n) -> p o n", o=4),
                            z4[:].rearrange("p (o n) -> p o n", o=4),
                            adj4,
                        )
                        for i, hh in enumerate(hs):
                            for c in range(2):
                                nc.tensor.matmul(
                                    acc[i][:, c * 512:(c + 1) * 512],
                                    wh_aug[jb][:, hh * 65:(hh + 1) * 65],
                                    x4[:, i * N + c * 512:i * N + (c + 1) * 512],
                                    start=(jb == 0), stop=(jb == NB - 1),
                                )
                        if qg == 1 and jb < 4:
                            h_prev, acc_prev = prev_acc.pop(jb)
                            epi_emit(h_prev, acc_prev)
                    if qg == 0:
                        prev_acc = {i: (hs[i], acc[i]) for i in range(4)}
                for i, hh in enumerate(hs):
                    epi_emit(hh, acc[i], dve=(i % 2 == 1),
                             dma_eng=nc.scalar if i % 2 == 0 else nc.sync)

    if split:
        _split_sync_waits(nc)
    return nc


_NC_CACHE = None


def _get_nc():
    global _NC_CACHE
    if _NC_CACHE is None:
        _NC_CACHE = build_nc()
    return _NC_CACHE


_NPDT = np.dtype(mybir.dt.np(FP16))


def _prep_in_maps(h, adj, W, a):
    h = np.asarray(h, dtype=np.float32)
    adj = np.asarray(adj)
    W = np.asarray(W, dtype=np.float32)
    a = np.asarray(a, dtype=np.float32)
    amat = np.zeros((FO, 2 * H), dtype=np.float32)
    for hh in range(H):
        amat[hh * FOH:(hh + 1) * FOH, hh] = a[hh, :FOH]
        amat[hh * FOH:(hh + 1) * FOH, H + hh] = a[hh, FOH:]
    wamat = W @ amat                       # [FIN, 16] fp32
    wp = np.ascontiguousarray(
        np.concatenate([W[0:128, :], W[128:256, :]], axis=1), dtype=_NPDT
    )                                      # [128, 1024] packed (2KB rows)
    in_maps = []
    for c in range(N_CORES):
        ee = (h[c] @ wamat).T              # [16, N] fp32: rows 0..7 es, 8..15 ed
        es, ed = ee[0:8], ee[8:16]
        mh = 0.8 * es.max(axis=1, keepdims=True)     # [8, 1]
        rr = np.zeros((16, N), dtype=_NPDT)
        rr[0:8] = np.exp(0.8 * es)
        vq = np.zeros((32, N), dtype=np.float32)
        vq[8:16] = np.exp(ed - mh)
        vq[24:32] = np.exp(ALPHA * ed - mh)
        in_maps.append({
            "hT": np.ascontiguousarray(h[c].T, dtype=_NPDT),
            "adjT": np.ascontiguousarray(adj[c].T, dtype=_NPDT),
            "Wp": wp,
            "R": rr,
            "VQ": vq,
        })
    return in_maps


def run(h, adj, W, a, trace=False, **kw):
    nc = _get_nc()
    in_maps = _prep_in_maps(h, adj, W, a)
    res = run_bass_kernel_spmd(nc, in_maps, list(range(N_CORES)), trace=trace, **kw)
    out = np.empty((N_CORES, N, FO), dtype=np.float32)
    for c in range(N_CORES):
        arr = res.results[c]["out"].reshape(H, 65, N)
        num = arr[:, :FOH, :]              # [H, 64, N]
        den = arr[:, FOH, :]               # [H, N]
        out[c] = (num / den[:, None, :]).transpose(2, 0, 1).reshape(N, FO)
    return out, res


def kernel(h, adj, W, a):
    out, _ = run(h, adj, W, a)
    return out
